# revision 1
# baseline (speedup 1.0000x reference)
"""DFlash draft-model loss/acc kernel for 8 Trainium2 NeuronCores.

Sharding: core c -> (batch b = c//4, query-quarter r = c%4).
Each core computes context features + K/V for its batch (bf16 matmuls,
fp32 accumulation), attention + LM head for its 512 draft rows over the
full vocab, and returns per-row stats (raw rowmax, raw label logit,
scaled sum-exp, rstd). The host computes the weighted CE loss and
accuracy from the stats.
"""
import math
import sys

sys.path.insert(0, "/opt/trn_rl_repo")

import numpy as np
import ml_dtypes

import concourse.bass as bass
import concourse.mybir as mybir
import concourse.tile as tile
from concourse import bacc

BF16 = ml_dtypes.bfloat16
F32 = mybir.dt.float32
BF = mybir.dt.bfloat16
AX = mybir.AxisListType
OP = mybir.AluOpType
ACTF = mybir.ActivationFunctionType

L, B, S, D = 3, 2, 2048, 1024
H, DH = 8, 128
NA, BS = 128, 16
Q = NA * BS            # 2048 draft tokens per batch
V = 32000
MASK_ID = V - 1
GAMMA, EPS = 7.0, 1e-6
NCORES, RPG = 8, 4     # 2 batch groups x 4 row-quarters
QL = Q // RPG          # 512 local draft rows per core
SCH = S // 512         # 4 ctx chunks of 512
KVW = (S + QL) // 128  # 20 kv wrap-blocks (16 ctx + 4 draft)
NEG = -1.0e30
NVCH = (V + 511) // 512            # 63 vocab chunks
VG = 2048                          # vocab staging group (cols)
NVG = (V + VG - 1) // VG           # 16 staging groups


def _wrap(x):
    # [K, N] row-major -> (128, K//128, N): [p, kb, n] = x[kb*128 + p, n]
    K, N = x.shape
    return np.ascontiguousarray(x.reshape(K // 128, 128, N).transpose(1, 0, 2))


def _bfw(x):
    return _wrap(np.asarray(x, np.float32)).astype(BF16)


def _rope_tables(pos):
    # pos: [n] int -> cos/sin [64, n] f32 (row j = dim j angle tables)
    inv = (1.0 / (10000.0 ** (np.arange(64, dtype=np.float32) / 64.0))).astype(np.float32)
    ang = inv[:, None] * pos[None, :].astype(np.float32)
    return np.cos(ang).astype(np.float32), np.sin(ang).astype(np.float32)


def host_prep(inputs):
    """Compute index/label/weight arrays and per-core device inputs."""
    ii = np.asarray(inputs["input_ids"]).astype(np.int64)
    anch = np.asarray(inputs["anchor_positions"]).astype(np.int64)
    hs = np.asarray(inputs["hidden_states"], np.float32)
    lmw = np.asarray(inputs["lm_head_weight"], np.float32)
    nw = np.asarray(inputs["norm_weight"], np.float32)
    fc = np.asarray(inputs["fc_weight"], np.float32)
    emb = np.asarray(inputs["embed_table"], np.float32)
    wq = np.asarray(inputs["wq"], np.float32)
    wk = np.asarray(inputs["wk"], np.float32)
    wv = np.asarray(inputs["wv"], np.float32)
    wo = np.asarray(inputs["wo"], np.float32)

    offs = np.arange(BS, dtype=np.int64)
    pos_flat = (anch[:, :, None] + offs[None, None, :]).reshape(B, Q)
    in_bounds = pos_flat < S
    gidx = np.minimum(pos_flat, S - 1)
    all_tok = np.take_along_axis(ii, gidx, axis=1)
    pos_in_block = np.arange(Q) % BS
    is_anchor = pos_in_block == 0
    draft_ids = np.where(is_anchor[None, :], all_tok, MASK_ID)
    labels = np.where((~is_anchor)[None, :] & in_bounds, all_tok, -100)
    lbl = np.maximum(labels, 0)
    anc_q = anch[:, np.arange(Q) // BS]          # [B, Q] anchor per draft row

    # shared (batch-independent) tensors
    sc_q = 1.0 / math.sqrt(DH)
    shared = {
        "fcT": _bfw(fc.T),                        # [3072 -> D] kxm
        "wqTs": _bfw(wq.T * sc_q),
        "wkT": _bfw(wk.T),
        "wvT": _bfw(wv.T),
        "woT": _bfw(wo.T),
        "lmT": _bfw((lmw * nw[None, :]).T),       # (128, 8, 32000)
        "ident_bf": np.eye(128, dtype=np.float32).astype(BF16),
        "ident_f32": np.eye(128, dtype=np.float32),
    }
    cosc, sinc = _rope_tables(np.arange(S))
    shared["cosc"], shared["sinc"] = cosc, sinc
    qi = np.arange(128)
    shared["dmask"] = np.where((qi[:, None] // BS) == (qi[None, :] // BS),
                               0.0, NEG).astype(np.float32)

    per_core = []
    for c in range(NCORES):
        b, r = c // RPG, c % RPG
        sl = slice(r * QL, (r + 1) * QL)
        hcat = hs[:, b].transpose(1, 0, 2).reshape(S, L * D)   # [S, 3072]
        embT = emb[draft_ids[b]].T                              # [D, Q]
        cosd, sind = _rope_tables(pos_flat[b, sl])
        kv = np.arange(S)
        mb = np.where(kv[None, :] < anc_q[b, sl][:, None], 0.0, NEG).astype(np.float32)
        d = dict(shared)
        d.update({
            "hcatT": _bfw(hcat.T),                              # (128,24,2048)
            "embT": _bfw(embT[:, sl]),                          # (128,8,512)
            "lblT": _bfw((lmw * nw[None, :])[lbl[b, sl]].T),    # (128,8,512)
            "maskb": _wrap(mb),                                 # (128,4,2048) f32
            "cosd": cosd, "sind": sind,                         # [64,512]
        })
        per_core.append(d)

    meta = dict(labels=labels, lbl=lbl, pos_in_block=pos_in_block)
    return per_core, meta


def host_reduce(stats_list, meta):
    """stats_list: per-core [4, 512] f32 rows (M0, labdot, sumexp, rstd)."""
    labels = meta["labels"]
    pib = meta["pos_in_block"]
    decay = np.concatenate([np.zeros(1, np.float32),
                            np.exp(-(np.arange(1, BS, dtype=np.float32) - 1.0) / GAMMA)])
    w_all = decay[pib][None, :] * (labels != -100).astype(np.float32)

    num = 0.0
    den = 0.0
    ncorr = 0
    nvalid = int((labels != -100).sum())
    for c in range(NCORES):
        b, r = c // RPG, c % RPG
        st = stats_list[c]
        m0, labd, sexp, rstd = st[0], st[1], st[2], st[3]
        nll = np.log(sexp) - rstd * labd
        w = w_all[b, r * QL:(r + 1) * QL]
        num += float((w * nll).sum())
        den += float(w.sum())
        valid = labels[b, r * QL:(r + 1) * QL] != -100
        ncorr += int(((labd >= m0) & valid).sum())
    loss = np.float32(num / max(den, 1e-6))
    acc = np.float32(ncorr / max(nvalid, 1))
    return loss, acc


_PROG = None


def _rope(nc, pool, dst, src_ps, cos, sin, n):
    """dst[0:64] = x1*cos - x2*sin ; dst[64:128] = x1*sin + x2*cos.
    src_ps: [128, n] psum f32; cos/sin: [64, n] sbuf f32; dst: [128, n] bf16."""
    t1 = pool.tile([64, n], F32, tag="rope_t1")
    t2 = pool.tile([64, n], F32, tag="rope_t2")
    x1, x2 = src_ps[0:64, :], src_ps[64:128, :]
    nc.vector.tensor_mul(t1[:], x1, cos[:])
    nc.vector.tensor_mul(t2[:], x2, sin[:])
    nc.vector.tensor_sub(dst[0:64, :], t1[:], t2[:])
    nc.vector.tensor_mul(t1[:], x1, sin[:])
    nc.vector.tensor_mul(t2[:], x2, cos[:])
    nc.vector.tensor_add(dst[64:128, :], t1[:], t2[:])


def build_program():
    global _PROG
    if _PROG is not None:
        return _PROG
    import os
    phases = os.environ.get("DFLASH_PHASES", "123")
    lmparts = os.environ.get("DFLASH_LM", "abc")
    nc = bacc.Bacc(None, target_bir_lowering=False, debug=False)
    names = {}
    with tile.TileContext(nc) as tc:
        with tc.tile_pool(name="dram", bufs=1, space="DRAM") as dram:
            def din(name, shape, dt=BF):
                t = dram.tile(shape, dt, kind="ExternalInput", name=name)
                names[name] = t.name
                return t

            hcatT = din("hcatT", [128, 24, 2048])
            fcT = din("fcT", [128, 24, 1024])
            wqTs = din("wqTs", [128, 8, 1024])
            wkT = din("wkT", [128, 8, 1024])
            wvT = din("wvT", [128, 8, 1024])
            woT = din("woT", [128, 8, 1024])
            lmT = din("lmT", [128, 8, V])
            embT = din("embT", [128, 8, QL])
            lblT = din("lblT", [128, 8, QL])
            maskb = din("maskb", [128, 4, 2048], F32)
            cosc = din("cosc", [64, S], F32)
            sinc = din("sinc", [64, S], F32)
            cosd = din("cosd", [64, QL], F32)
            sind = din("sind", [64, QL], F32)
            dmask = din("dmask", [128, 128], F32)
            ident_bf = din("ident_bf", [128, 128])
            ident_f32 = din("ident_f32", [128, 128], F32)

            stats = dram.tile([4, QL], F32, kind="ExternalOutput", name="stats")
            names["stats"] = stats.name

            kT_d = dram.tile([128, 8, S + QL], BF, name="kT_scratch")
            v_d = dram.tile([128, KVW, 1024], BF, name="v_scratch")
            den_d = dram.tile([H, QL], F32, name="den_scratch")
            rstd_d = dram.tile([QL], F32, name="rstd_scratch")

            import contextlib
            with contextlib.ExitStack() as ctx:
                # psum pools shared across phases (<= 8 banks total)
                ps_big = ctx.enter_context(tc.tile_pool(name="ps_big", bufs=4, space="PSUM"))
                ps_tr = ctx.enter_context(tc.tile_pool(name="ps_tr", bufs=2, space="PSUM"))
                ps_row = ctx.enter_context(tc.tile_pool(name="ps_row", bufs=2, space="PSUM"))
                persist = ctx.enter_context(tc.tile_pool(name="persist", bufs=1))

                qTr = persist.tile([128, 8, QL], BF)        # roped q, feature-major
                embT_sb = persist.tile([128, 8, QL], BF)
                hbf = persist.tile([128, 8, QL], BF)
                ones_bf = persist.tile([128, 1], BF)
                ones_f32 = persist.tile([128, 1], F32)
                eps_t = persist.tile([1, 1], F32)
                nc.vector.memset(ones_bf[:], 1.0)
                nc.vector.memset(ones_f32[:], 1.0)
                nc.vector.memset(eps_t[:], EPS)
                nc.sync.dma_start(out=embT_sb[:], in_=embT[:])

                # ---------- phase 1: draft projections + ctx K/V ----------
                if "1" in phases:
                  with tc.tile_pool(name="ph1", bufs=2) as ph1, \
                     tc.tile_pool(name="ph1w", bufs=1) as ph1w:
                    wq_sb = ph1w.tile([128, 8, 1024], BF)
                    wk_sb = ph1w.tile([128, 8, 1024], BF)
                    wv_sb = ph1w.tile([128, 8, 1024], BF)
                    cosd_sb = ph1w.tile([64, QL], F32)
                    sind_sb = ph1w.tile([64, QL], F32)
                    cosc_sb = ph1w.tile([64, S], F32)
                    sinc_sb = ph1w.tile([64, S], F32)
                    nc.sync.dma_start(out=wq_sb[:], in_=wqTs[:])
                    nc.sync.dma_start(out=wk_sb[:], in_=wkT[:])
                    nc.sync.dma_start(out=wv_sb[:], in_=wvT[:])
                    nc.sync.dma_start(out=cosd_sb[:], in_=cosd[:])
                    nc.sync.dma_start(out=sind_sb[:], in_=sind[:])
                    nc.sync.dma_start(out=cosc_sb[:], in_=cosc[:])
                    nc.sync.dma_start(out=sinc_sb[:], in_=sinc[:])

                    # draft q/k (feature-major, roped) and v (token-major)
                    for mb in range(8):
                        qp = ps_big.tile([128, QL], F32, tag="mm")
                        for kb in range(8):
                            nc.tensor.matmul(qp[:], wq_sb[:, kb, mb * 128:(mb + 1) * 128],
                                             embT_sb[:, kb, :], start=kb == 0, stop=kb == 7)
                        _rope(nc, ph1, qTr[:, mb, :], qp, cosd_sb, sind_sb, QL)
                    for mb in range(8):
                        kp = ps_big.tile([128, QL], F32, tag="mm")
                        for kb in range(8):
                            nc.tensor.matmul(kp[:], wk_sb[:, kb, mb * 128:(mb + 1) * 128],
                                             embT_sb[:, kb, :], start=kb == 0, stop=kb == 7)
                        kd_sb = ph1.tile([128, QL], BF, tag="kd")
                        _rope(nc, ph1, kd_sb[:], kp, cosd_sb, sind_sb, QL)
                        nc.sync.dma_start(out=kT_d[:, mb, S:S + QL], in_=kd_sb[:])
                    for sm in range(4):
                        for nn2 in range(2):
                            vp = ps_big.tile([128, 512], F32, tag="mm")
                            for kb in range(8):
                                nc.tensor.matmul(vp[:], embT_sb[:, kb, sm * 128:(sm + 1) * 128],
                                                 wv_sb[:, kb, nn2 * 512:(nn2 + 1) * 512],
                                                 start=kb == 0, stop=kb == 7)
                            vd_sb = ph1.tile([128, 512], BF, tag="vd")
                            nc.vector.tensor_copy(vd_sb[:], vp[:])
                            nc.sync.dma_start(out=v_d[:, 16 + sm, nn2 * 512:(nn2 + 1) * 512],
                                              in_=vd_sb[:])

                    # ctx chunks: ctxT -> kcT (roped) + vc
                    for sc in range(SCH):
                        ssl = slice(sc * 512, (sc + 1) * 512)
                        hc_sb = ph1.tile([128, 24, 512], BF, tag="hcat")
                        nc.sync.dma_start(out=hc_sb[:], in_=hcatT[:, :, ssl])
                        ctx_sb = ph1.tile([128, 8, 512], BF, tag="ctx")
                        for mb in range(8):
                            fcmb = ph1.tile([128, 24, 128], BF, tag="fcmb")
                            nc.sync.dma_start(out=fcmb[:], in_=fcT[:, :, mb * 128:(mb + 1) * 128])
                            cp = ps_big.tile([128, 512], F32, tag="mm")
                            for kb in range(24):
                                nc.tensor.matmul(cp[:], fcmb[:, kb, :],
                                                 hc_sb[:, kb, :], start=kb == 0, stop=kb == 23)
                            nc.vector.tensor_copy(ctx_sb[:, mb, :], cp[:])
                        for mb in range(8):
                            kp = ps_big.tile([128, 512], F32, tag="mm")
                            for kb in range(8):
                                nc.tensor.matmul(kp[:], wk_sb[:, kb, mb * 128:(mb + 1) * 128],
                                                 ctx_sb[:, kb, :], start=kb == 0, stop=kb == 7)
                            kc_sb = ph1.tile([128, 512], BF, tag="kc")
                            _rope(nc, ph1, kc_sb[:], kp, cosc_sb[:, ssl], sinc_sb[:, ssl], 512)
                            nc.sync.dma_start(out=kT_d[:, mb, ssl], in_=kc_sb[:])
                        for sm in range(4):
                            for nn2 in range(2):
                                vp = ps_big.tile([128, 512], F32, tag="mm")
                                for kb in range(8):
                                    nc.tensor.matmul(vp[:], ctx_sb[:, kb, sm * 128:(sm + 1) * 128],
                                                     wv_sb[:, kb, nn2 * 512:(nn2 + 1) * 512],
                                                     start=kb == 0, stop=kb == 7)
                                vc_sb = ph1.tile([128, 512], BF, tag="vc")
                                nc.vector.tensor_copy(vc_sb[:], vp[:])
                                nc.sync.dma_start(out=v_d[:, sc * 4 + sm, nn2 * 512:(nn2 + 1) * 512],
                                                  in_=vc_sb[:])

                # ---------- phase 2: attention ----------
                if "2" in phases:
                  with tc.tile_pool(name="ph2", bufs=2) as ph2, \
                     tc.tile_pool(name="ph2c", bufs=1) as ph2c, \
                     tc.tile_pool(name="ph2p", bufs=3) as ph2p:
                    maskb_sb = ph2c.tile([128, 4, 2048], F32)
                    nc.sync.dma_start(out=maskb_sb[:], in_=maskb[:])
                    dmask_sb = ph2c.tile([128, 128], F32)
                    nc.sync.dma_start(out=dmask_sb[:], in_=dmask[:])
                    idb_sb = ph2c.tile([128, 128], BF)
                    nc.sync.dma_start(out=idb_sb[:], in_=ident_bf[:])
                    wo_sb = ph2c.tile([128, 8, 1024], BF)
                    nc.sync.dma_start(out=wo_sb[:], in_=woT[:])
                    outT = ph2c.tile([128, 8, QL], BF)
                    h_sb = ph2c.tile([128, 8, QL], F32)

                    for h in range(H):
                        kh_sb = ph2.tile([128, S + QL], BF, tag="kh")
                        nc.sync.dma_start(out=kh_sb[:], in_=kT_d[:, h, :])
                        vh_sb = ph2.tile([128, KVW, 128], BF, tag="vh")
                        nc.sync.dma_start(out=vh_sb[:], in_=v_d[:, :, h * 128:(h + 1) * 128])
                        pT = ph2.tile([128, KVW, QL], BF, tag="pT")
                        nc.vector.memset(pT[:, 16:20, :], 0.0)
                        for qt in range(4):
                            qsl = slice(qt * 128, (qt + 1) * 128)
                            for cch in range(SCH):
                                sp = ps_big.tile([128, 512], F32, tag="mm")
                                nc.tensor.matmul(sp[:], qTr[:, h, qsl],
                                                 kh_sb[:, cch * 512:(cch + 1) * 512])
                                sm_sb = ph2p.tile([128, 512], F32, tag="smask")
                                nc.vector.tensor_add(sm_sb[:], sp[:],
                                                     maskb_sb[:, qt, cch * 512:(cch + 1) * 512])
                                pr_sb = ph2p.tile([128, 512], BF, tag="probs")
                                nc.scalar.activation(pr_sb[:], sm_sb[:], ACTF.Exp)
                                for j in range(4):
                                    trp = ps_tr.tile([128, 128], BF, tag="small")
                                    nc.tensor.transpose(trp[:], pr_sb[:, j * 128:(j + 1) * 128],
                                                        idb_sb[:])
                                    nc.vector.tensor_copy(pT[:, cch * 4 + j, qsl], trp[:])
                            # draft block: bidirectional within own 128 range
                            sp = ps_tr.tile([128, 128], F32, tag="small")
                            nc.tensor.matmul(sp[:], qTr[:, h, qsl], kh_sb[:, S + qt * 128:S + (qt + 1) * 128])
                            smd = ph2p.tile([128, 128], F32, tag="smaskd")
                            nc.vector.tensor_add(smd[:], sp[:], dmask_sb[:])
                            prd = ph2p.tile([128, 128], BF, tag="probsd")
                            nc.scalar.activation(prd[:], smd[:], ACTF.Exp)
                            trp = ps_tr.tile([128, 128], BF, tag="small")
                            nc.tensor.transpose(trp[:], prd[:], idb_sb[:])
                            nc.vector.tensor_copy(pT[:, 16 + qt, qsl], trp[:])

                        dp = ps_row.tile([1, QL], F32, tag="row")
                        for cc in range(KVW):
                            nc.tensor.matmul(dp[:], ones_bf[:], pT[:, cc, :],
                                             start=cc == 0, stop=cc == KVW - 1)
                        den_sb = ph2.tile([1, QL], F32, tag="den_sb")
                        nc.vector.reciprocal(den_sb[:], dp[:])
                        nc.sync.dma_start(out=den_d[h, :], in_=den_sb[:])
                        den_bc = ph2.tile([128, QL], F32, tag="den_bc")
                        row = den_d[h:h + 1, :]
                        bc_ap = bass.AP(tensor=row.tensor, offset=row.offset,
                                        ap=[[0, 128]] + list(row.ap)[1:])
                        nc.sync.dma_start(out=den_bc[:], in_=bc_ap)

                        ap_ps = ps_big.tile([128, QL], F32, tag="mm")
                        for cc in range(KVW):
                            nc.tensor.matmul(ap_ps[:], vh_sb[:, cc, :], pT[:, cc, :],
                                             start=cc == 0, stop=cc == KVW - 1)
                        nc.vector.tensor_mul(outT[:, h, :], ap_ps[:], den_bc[:])

                    # wo projection + residual + sumsq/rstd
                    sqp = ps_row.tile([1, QL], F32, tag="row")
                    for mb in range(8):
                        op = ps_big.tile([128, QL], F32, tag="mm")
                        for kb in range(8):
                            nc.tensor.matmul(op[:], wo_sb[:, kb, mb * 128:(mb + 1) * 128],
                                             outT[:, kb, :], start=kb == 0, stop=kb == 7)
                        nc.vector.tensor_add(h_sb[:, mb, :], op[:], embT_sb[:, mb, :])
                        nc.vector.tensor_copy(hbf[:, mb, :], h_sb[:, mb, :])
                        hsq = ph2.tile([128, QL], F32, tag="hsq")
                        nc.vector.tensor_mul(hsq[:], h_sb[:, mb, :], h_sb[:, mb, :])
                        nc.tensor.matmul(sqp[:], ones_f32[:], hsq[:],
                                         start=mb == 0, stop=mb == 7)
                    sq_sb = ph2.tile([1, QL], F32, tag="sq_sb")
                    nc.scalar.activation(sq_sb[:], sqp[:], ACTF.Sqrt,
                                         bias=eps_t[0:1, :], scale=1.0 / D)
                    rstd_sb = ph2.tile([1, QL], F32, tag="rstd_sb")
                    nc.vector.reciprocal(rstd_sb[:], sq_sb[:])
                    nc.sync.dma_start(out=rstd_d[:], in_=rstd_sb[0:1, :])
                    nc.sync.dma_start(out=stats[3:4, :], in_=rstd_sb[:])

                # ---------- phase 3: LM head ----------
                if "3" in phases:
                  with tc.tile_pool(name="ph3", bufs=2) as ph3, \
                     tc.tile_pool(name="ph3c", bufs=1) as ph3c, \
                     tc.tile_pool(name="ph3a", bufs=1) as ph3a:
                    lbl_sb = ph3c.tile([128, 8, QL], BF)
                    nc.sync.dma_start(out=lbl_sb[:], in_=lblT[:])
                    idf_sb = ph3c.tile([128, 128], F32)
                    nc.sync.dma_start(out=idf_sb[:], in_=ident_f32[:])
                    m0 = [ph3a.tile([128, 1], F32, name=f"m0_{qt}") for qt in range(4)]
                    se = [ph3a.tile([128, 1], F32, name=f"se_{qt}") for qt in range(4)]
                    rstd_pt = [ph3a.tile([128, 1], F32, name=f"rstd_pt_{qt}") for qt in range(4)]
                    for qt in range(4):
                        nc.vector.memset(m0[qt][:], -3.0e38)
                        nc.vector.memset(se[qt][:], 0.0)
                        rsl = rstd_d[qt * 128:(qt + 1) * 128]
                        nc.sync.dma_start(out=rstd_pt[qt][:], in_=rsl)

                    for vg in range(NVG):
                        ncol = min(VG, V - vg * VG)
                        lmt = ph3.tile([128, 8, VG], BF, tag="lmt")
                        nc.sync.dma_start(out=lmt[:, :, :ncol], in_=lmT[:, :, vg * VG:vg * VG + ncol])
                        for qt in range(4):
                            qsl = slice(qt * 128, (qt + 1) * 128)
                            nch = (ncol + 511) // 512
                            pss = [ps_big.tile([128, 512], F32, tag="mm", name=f"lp_{vg}_{qt}_{i}")
                                   for i in range(nch)]
                            for kb in range(8):
                                for cc in range(nch):
                                    w = min(512, ncol - cc * 512)
                                    nc.tensor.matmul(pss[cc][:, :w], hbf[:, kb, qsl],
                                                     lmt[:, kb, cc * 512:cc * 512 + w],
                                                     start=kb == 0, stop=kb == 7)
                            for cc in range(nch):
                                w = min(512, ncol - cc * 512)
                                lp = pss[cc]
                                if "a" in lmparts:
                                    cmax = ph3.tile([128, 1], F32, tag="cmax")
                                    nc.vector.tensor_reduce(cmax[:], lp[:, :w], axis=AX.X, op=OP.max)
                                    nc.vector.tensor_max(m0[qt][:], m0[qt][:], cmax[:])
                                if "b" in lmparts:
                                    ej = ph3.tile([128, 512], BF, tag="ej")
                                    csum = ph3.tile([128, 1], F32, tag="csum")
                                    nc.scalar.activation(ej[:, :w], lp[:, :w], ACTF.Exp,
                                                         scale=rstd_pt[qt][:], accum_out=csum[:])
                                    nc.vector.tensor_add(se[qt][:], se[qt][:], csum[:])

                    for qt in range(4):
                        qsl = slice(qt * 128, (qt + 1) * 128)
                        if "c" in lmparts:
                            lpp = ps_tr.tile([128, 128], F32, tag="small")
                            for kb in range(8):
                                nc.tensor.matmul(lpp[:], hbf[:, kb, qsl], lbl_sb[:, kb, qsl],
                                                 start=kb == 0, stop=kb == 7)
                            junk = ph3.tile([128, 128], F32, tag="junk")
                            labd = ph3.tile([128, 1], F32, tag="labd")
                            nc.vector.tensor_mul(junk[:], lpp[:], idf_sb[:])
                            nc.vector.tensor_reduce(labd[:], junk[:], axis=AX.X, op=OP.add)
                            nc.sync.dma_start(out=stats[1:2, qsl], in_=labd[:])
                        if "a" in lmparts:
                            nc.sync.dma_start(out=stats[0:1, qsl], in_=m0[qt][:])
                        if "b" in lmparts:
                            nc.sync.dma_start(out=stats[2:3, qsl], in_=se[qt][:])
    nc.compile()
    _PROG = (nc, names)
    return _PROG


def kernel(**inputs):
    import os
    from concourse.bass_utils import run_bass_kernel_spmd
    nc, names = build_program()
    per_core, meta = host_prep(inputs)
    in_maps = []
    for c in range(NCORES):
        m = {}
        for k, arr in per_core[c].items():
            dt = BF16 if arr.dtype == BF16 else np.float32
            m[names[k]] = np.ascontiguousarray(arr, dtype=dt)
        in_maps.append(m)
    trace = bool(os.environ.get("DFLASH_TRACE"))
    if trace:
        try:
            import ntff_shim
            ntff_shim.install()
        except Exception:
            trace = False
    res = run_bass_kernel_spmd(nc, in_maps, list(range(NCORES)), trace=trace)
    kernel.last_exec_ns = res.exec_time_ns
    stats_list = [res.results[c][names["stats"]] for c in range(NCORES)]
    return host_reduce(stats_list, meta)



# revision 4
# speedup vs baseline: 1.9539x; 1.9539x over previous
"""DFlash draft-model loss/acc kernel for 8 Trainium2 NeuronCores.

Sharding: core c -> (batch b = c//4, rank r = c%4). Each core computes
context features + roped K / V for context chunk r (512 of 2048 tokens)
and the draft q/k/v for its own 512 draft rows, AllGathers the context
K/V across its 4-core batch group, then runs kv-major attention + the
fp8 LM head over the full vocab for its 512 rows. Host combines per-row
stats (max-exp, label logit, sum-exp, rstd) into the loss/acc.
"""
import math
import sys

sys.path.insert(0, "/opt/trn_rl_repo")

import numpy as np
import ml_dtypes

import concourse.bass as bass
import concourse.mybir as mybir
import concourse.tile as tile
from concourse import bacc

BF16 = ml_dtypes.bfloat16
F8E4 = ml_dtypes.float8_e4m3
F32 = mybir.dt.float32
BF = mybir.dt.bfloat16
FP8 = mybir.dt.float8e4
AX = mybir.AxisListType
OP = mybir.AluOpType
ACTF = mybir.ActivationFunctionType
DR = mybir.MatmulPerfMode.DoubleRow

L, B, S, D = 3, 2, 2048, 1024
H, DH = 8, 128
NA, BS = 128, 16
Q = NA * BS            # 2048 draft tokens per batch
V = 32000
MASK_ID = V - 1
GAMMA, EPS = 7.0, 1e-6
NCORES, RPG = 8, 4     # 2 batch groups x 4 ranks
QL = Q // RPG          # 512 local draft rows per core
CH = S // RPG          # 512 ctx tokens per core
NEG = -1.0e30
VG = 2048              # vocab staging group (cols)
NVG = (V + VG - 1) // VG
SH = 32.0              # fp8 scale for hidden-side operands
SW = 2048.0            # fp8 scale for big weight matrices
SQ = 256.0             # fp8 scale for square qkv weights
DS_CTX = 1.0 / (SH * SW)        # 2^-16
DS_QKV = 1.0 / (SH * SQ)        # 2^-13
DS_LM = 1.0 / (SH * SW)         # 2^-16


def _wrap(x):
    # [K, N] row-major -> (128, K//128, N): [p, kb, n] = x[kb*128 + p, n]
    K, N = x.shape
    return np.ascontiguousarray(x.reshape(K // 128, 128, N).transpose(1, 0, 2))


def _bfw(x):
    return _wrap(np.asarray(x, np.float32)).astype(BF16)


def _f8w(x, s):
    q = np.clip(np.asarray(x, np.float32) * s, -448.0, 448.0)
    return _wrap(q).astype(F8E4)


def _rope_tables(pos):
    # pos: [n] int -> cos/sin [64, n] f32 (row j = dim j angle tables)
    inv = (1.0 / (10000.0 ** (np.arange(64, dtype=np.float32) / 64.0))).astype(np.float32)
    ang = inv[:, None] * pos[None, :].astype(np.float32)
    return np.cos(ang).astype(np.float32), np.sin(ang).astype(np.float32)


def host_prep(inputs):
    """Compute index/label/weight arrays and per-core device inputs."""
    ii = np.asarray(inputs["input_ids"]).astype(np.int64)
    anch = np.asarray(inputs["anchor_positions"]).astype(np.int64)
    hs = np.asarray(inputs["hidden_states"], np.float32)
    lmw = np.asarray(inputs["lm_head_weight"], np.float32)
    nw = np.asarray(inputs["norm_weight"], np.float32)
    fc = np.asarray(inputs["fc_weight"], np.float32)
    emb = np.asarray(inputs["embed_table"], np.float32)
    wq = np.asarray(inputs["wq"], np.float32)
    wk = np.asarray(inputs["wk"], np.float32)
    wv = np.asarray(inputs["wv"], np.float32)
    wo = np.asarray(inputs["wo"], np.float32)

    offs = np.arange(BS, dtype=np.int64)
    pos_flat = (anch[:, :, None] + offs[None, None, :]).reshape(B, Q)
    in_bounds = pos_flat < S
    gidx = np.minimum(pos_flat, S - 1)
    all_tok = np.take_along_axis(ii, gidx, axis=1)
    pos_in_block = np.arange(Q) % BS
    is_anchor = pos_in_block == 0
    draft_ids = np.where(is_anchor[None, :], all_tok, MASK_ID)
    labels = np.where((~is_anchor)[None, :] & in_bounds, all_tok, -100)
    lbl = np.maximum(labels, 0)
    anc_q = anch[:, np.arange(Q) // BS]          # [B, Q] anchor per draft row

    lmn = lmw * nw[None, :]
    shared = {
        "fc8": _f8w(fc.T, SW),                    # [128, 24, 1024]
        "wq8": _f8w(wq.T, SQ),
        "wk8": _f8w(wk.T, SQ),
        "wv8": _f8w(wv.T, SQ),
        "wkc": _bfw(wk.T * DS_CTX),
        "wvc": _bfw(wv.T * DS_CTX),
        "woT": _bfw(wo.T),
        "lm8": _f8w(lmn.T, SW),                   # (128, 8, 32000)
        "ident_f32": np.eye(128, dtype=np.float32),
    }
    qi = np.arange(128)
    shared["dmask"] = np.where((qi[:, None] // BS) == (qi[None, :] // BS),
                               0.0, NEG).astype(np.float32)

    sc_q = 1.0 / math.sqrt(DH)
    per_core = []
    for c in range(NCORES):
        b, r = c // RPG, c % RPG
        sl = slice(r * QL, (r + 1) * QL)
        csl = slice(r * CH, (r + 1) * CH)
        hcat = hs[:, b, csl].transpose(1, 0, 2).reshape(CH, L * D)  # [512, 3072]
        embT = emb[draft_ids[b, sl]].T                              # [D, 512]
        cosd, sind = _rope_tables(pos_flat[b, sl])
        cosc, sinc = _rope_tables(np.arange(r * CH, (r + 1) * CH))
        # kv-major ctx mask for this core's rows: [p, cc, j]
        kvp = np.arange(S).reshape(16, 128).T                       # [128, 16]
        mT = np.where(kvp[:, :, None] < anc_q[b, sl][None, None, :],
                      0.0, NEG).astype(BF16)                        # [128,16,512]
        d = dict(shared)
        d.update({
            "hc8": _f8w(hcat.T, SH),                            # (128,24,512)
            "emb8": _f8w(embT, SH),                             # (128,8,512)
            "embT": _bfw(embT),                                 # (128,8,512)
            "lbl8": _f8w(lmn[lbl[b, sl]].T, SW),                # (128,8,512)
            "maskT": mT,
            "cosq": cosd * (DS_QKV * sc_q), "sinq": sind * (DS_QKV * sc_q),
            "cosk": cosd * DS_QKV, "sink": sind * DS_QKV,
            "cosc": cosc, "sinc": sinc,
        })
        per_core.append(d)

    meta = dict(labels=labels, lbl=lbl, pos_in_block=pos_in_block)
    return per_core, meta


def host_reduce(stats_list, meta):
    """stats_list: per-core [4, 512] f32 rows (m0e, labd_scaled, sumexp, rstd)."""
    labels = meta["labels"]
    pib = meta["pos_in_block"]
    decay = np.concatenate([np.zeros(1, np.float32),
                            np.exp(-(np.arange(1, BS, dtype=np.float32) - 1.0) / GAMMA)])
    w_all = decay[pib][None, :] * (labels != -100).astype(np.float32)

    num = 0.0
    den = 0.0
    ncorr = 0
    nvalid = int((labels != -100).sum())
    for c in range(NCORES):
        b, r = c // RPG, c % RPG
        st = np.asarray(stats_list[c], np.float64)
        m0e, labd_s, sexp, rstd = st[0], st[1], st[2], st[3]
        labd = labd_s * DS_LM
        nll = np.log(sexp) - rstd * labd
        w = w_all[b, r * QL:(r + 1) * QL]
        num += float((w * nll).sum())
        den += float(w.sum())
        valid = labels[b, r * QL:(r + 1) * QL] != -100
        ncorr += int(((np.exp(rstd * labd) >= m0e * 0.995) & valid).sum())
    loss = np.float32(num / max(den, 1e-6))
    acc = np.float32(ncorr / max(nvalid, 1))
    return loss, acc


_PROG = None


def _rope(nc, pool, dst, src_ps, cos, sin, n):
    """dst[0:64] = x1*cos - x2*sin ; dst[64:128] = x1*sin + x2*cos.
    src_ps: [128, n] psum f32; cos/sin: [64, n] sbuf f32; dst: [128, n] bf16."""
    t1 = pool.tile([64, n], F32, tag="rope_t1")
    t2 = pool.tile([64, n], F32, tag="rope_t2")
    x1, x2 = src_ps[0:64, :], src_ps[64:128, :]
    nc.vector.tensor_mul(t1[:], x1, cos[:])
    nc.vector.tensor_mul(t2[:], x2, sin[:])
    nc.vector.tensor_sub(dst[0:64, :], t1[:], t2[:])
    nc.vector.tensor_mul(t1[:], x1, sin[:])
    nc.vector.tensor_mul(t2[:], x2, cos[:])
    nc.vector.tensor_add(dst[64:128, :], t1[:], t2[:])


def build_program():
    global _PROG
    if _PROG is not None:
        return _PROG
    import contextlib
    nc = bacc.Bacc(None, target_bir_lowering=False, debug=False)
    names = {}
    with tile.TileContext(nc) as tc:
        with tc.tile_pool(name="dram", bufs=1, space="DRAM") as dram:
            def din(name, shape, dt=BF):
                t = dram.tile(shape, dt, kind="ExternalInput", name=name)
                names[name] = t.name
                return t

            hc8 = din("hc8", [128, 24, CH], FP8)
            fc8 = din("fc8", [128, 24, 1024], FP8)
            wq8 = din("wq8", [128, 8, 1024], FP8)
            wk8 = din("wk8", [128, 8, 1024], FP8)
            wv8 = din("wv8", [128, 8, 1024], FP8)
            wkc = din("wkc", [128, 8, 1024])
            wvc = din("wvc", [128, 8, 1024])
            woT = din("woT", [128, 8, 1024])
            lm8 = din("lm8", [128, 8, V], FP8)
            emb8 = din("emb8", [128, 8, QL], FP8)
            embT = din("embT", [128, 8, QL])
            lbl8 = din("lbl8", [128, 8, QL], FP8)
            maskT = din("maskT", [128, 16, QL])
            cosq = din("cosq", [64, QL], F32)
            sinq = din("sinq", [64, QL], F32)
            cosk = din("cosk", [64, QL], F32)
            sink = din("sink", [64, QL], F32)
            cosc = din("cosc", [64, CH], F32)
            sinc = din("sinc", [64, CH], F32)
            dmask = din("dmask", [128, 128], F32)
            ident_f32 = din("ident_f32", [128, 128], F32)

            stats = dram.tile([4, QL], F32, kind="ExternalOutput", name="stats")
            names["stats"] = stats.name

            # collective bounce: [p, sub(16), 512]; sub 0..7 = kcT per head,
            # sub 8..15 = v per head as [wrap(4) x feat(128)]
            kvcc_in = dram.tile([128, 16, 512], BF, name="kvcc_in")
            kvcc_out = dram.tile([RPG, 128, 16, 512], BF, name="kvcc_out")
            den_d = dram.tile([H, QL], F32, name="den_scratch")
            rstd_d = dram.tile([QL], F32, name="rstd_scratch")

            with contextlib.ExitStack() as ctx:
                ps_a = ctx.enter_context(tc.tile_pool(name="ps_a", bufs=4, space="PSUM"))
                ps_b = ctx.enter_context(tc.tile_pool(name="ps_b", bufs=2, space="PSUM"))
                ps_tr = ctx.enter_context(tc.tile_pool(name="ps_tr", bufs=1, space="PSUM"))
                ps_row = ctx.enter_context(tc.tile_pool(name="ps_row", bufs=1, space="PSUM"))
                persist = ctx.enter_context(tc.tile_pool(name="persist", bufs=1))

                qTr = persist.tile([128, 8, QL], BF)        # roped q, feature-major
                kdT = persist.tile([128, 8, QL], BF)        # roped draft k
                vd_sb = persist.tile([128, 8, 4, 128], BF)  # draft v [p,h,wrap,feat]
                embT_sb = persist.tile([128, 8, QL], BF)
                hq = persist.tile([128, 8, QL], FP8)        # h * SH quantized
                ones_bf = persist.tile([128, 1], BF)
                ones_f32 = persist.tile([128, 1], F32)
                eps_t = persist.tile([1, 1], F32)
                nc.vector.memset(ones_bf[:], 1.0)
                nc.vector.memset(ones_f32[:], 1.0)
                nc.vector.memset(eps_t[:], EPS)
                nc.sync.dma_start(out=embT_sb[:], in_=embT[:])

                # ---------- phase 1: ctx chunk K/V + draft projections ----------
                with tc.tile_pool(name="ph1", bufs=2) as ph1, \
                     tc.tile_pool(name="ph1w", bufs=1) as ph1w:
                    fc_sb = ph1w.tile([128, 24, 1024], FP8)
                    hc_sb = ph1w.tile([128, 24, CH], FP8)
                    wq_sb = ph1w.tile([128, 8, 1024], FP8)
                    wk_sb = ph1w.tile([128, 8, 1024], FP8)
                    wv_sb = ph1w.tile([128, 8, 1024], FP8)
                    wkc_sb = ph1w.tile([128, 8, 1024], BF)
                    wvc_sb = ph1w.tile([128, 8, 1024], BF)
                    emb8_sb = ph1w.tile([128, 8, QL], FP8)
                    cosq_sb = ph1w.tile([64, QL], F32)
                    sinq_sb = ph1w.tile([64, QL], F32)
                    cosk_sb = ph1w.tile([64, QL], F32)
                    sink_sb = ph1w.tile([64, QL], F32)
                    cosc_sb = ph1w.tile([64, CH], F32)
                    sinc_sb = ph1w.tile([64, CH], F32)
                    for dst, src in [(fc_sb, fc8), (hc_sb, hc8), (wq_sb, wq8),
                                     (wk_sb, wk8), (wv_sb, wv8), (wkc_sb, wkc),
                                     (wvc_sb, wvc), (emb8_sb, emb8),
                                     (cosq_sb, cosq), (sinq_sb, sinq),
                                     (cosk_sb, cosk), (sink_sb, sink),
                                     (cosc_sb, cosc), (sinc_sb, sinc)]:
                        nc.sync.dma_start(out=dst[:], in_=src[:])

                    # ctx features for chunk r (feature-major, scaled by 2^16)
                    ctx_sb = ph1w.tile([128, 8, CH], BF)
                    for mb in range(8):
                        cp = ps_a.tile([128, CH], F32, tag="mm")
                        for k2 in range(12):
                            nc.tensor.matmul(cp[:], fc_sb[:, 2 * k2:2 * k2 + 2, mb * 128:(mb + 1) * 128],
                                             hc_sb[:, 2 * k2:2 * k2 + 2, :],
                                             start=k2 == 0, stop=k2 == 11, perf_mode=DR)
                        nc.vector.tensor_copy(ctx_sb[:, mb, :], cp[:])

                    # ctx K (roped) -> bounce sub mb
                    for mb in range(8):
                        kp = ps_a.tile([128, CH], F32, tag="mm")
                        for kb in range(8):
                            nc.tensor.matmul(kp[:], wkc_sb[:, kb, mb * 128:(mb + 1) * 128],
                                             ctx_sb[:, kb, :], start=kb == 0, stop=kb == 7)
                        kc_sb = ph1.tile([128, CH], BF, tag="kc")
                        _rope(nc, ph1, kc_sb[:], kp, cosc_sb, sinc_sb, CH)
                        nc.sync.dma_start(out=kvcc_in[:, mb, :], in_=kc_sb[:])

                    # ctx V -> bounce subs 8..15 ([p, h, wrap*128+feat])
                    vc_stage = ph1w.tile([128, 8, CH], BF)
                    for sm in range(4):
                        for nn2 in range(2):
                            vp = ps_a.tile([128, 512], F32, tag="mm")
                            for kb in range(8):
                                nc.tensor.matmul(vp[:], ctx_sb[:, kb, sm * 128:(sm + 1) * 128],
                                                 wvc_sb[:, kb, nn2 * 512:(nn2 + 1) * 512],
                                                 start=kb == 0, stop=kb == 7)
                            for j in range(4):
                                nc.vector.tensor_copy(
                                    vc_stage[:, 4 * nn2 + j, sm * 128:(sm + 1) * 128],
                                    vp[:, j * 128:(j + 1) * 128])
                    nc.sync.dma_start(out=kvcc_in[:, 8:16, :], in_=vc_stage[:])

                    nc.gpsimd.collective_compute(
                        "AllGather", OP.bypass,
                        replica_groups=[[0, 1, 2, 3], [4, 5, 6, 7]],
                        ins=[kvcc_in.opt()], outs=[kvcc_out.opt()])

                    # draft q/k (roped, feature-major) and v (overlap collective)
                    for mb in range(8):
                        qp = ps_a.tile([128, QL], F32, tag="mm")
                        for k2 in range(4):
                            nc.tensor.matmul(qp[:], wq_sb[:, 2 * k2:2 * k2 + 2, mb * 128:(mb + 1) * 128],
                                             emb8_sb[:, 2 * k2:2 * k2 + 2, :],
                                             start=k2 == 0, stop=k2 == 3, perf_mode=DR)
                        _rope(nc, ph1, qTr[:, mb, :], qp, cosq_sb, sinq_sb, QL)
                    for mb in range(8):
                        kp = ps_a.tile([128, QL], F32, tag="mm")
                        for k2 in range(4):
                            nc.tensor.matmul(kp[:], wk_sb[:, 2 * k2:2 * k2 + 2, mb * 128:(mb + 1) * 128],
                                             emb8_sb[:, 2 * k2:2 * k2 + 2, :],
                                             start=k2 == 0, stop=k2 == 3, perf_mode=DR)
                        _rope(nc, ph1, kdT[:, mb, :], kp, cosk_sb, sink_sb, QL)
                    for sm in range(4):
                        for nn2 in range(2):
                            vp = ps_a.tile([128, 512], F32, tag="mm")
                            for k2 in range(4):
                                nc.tensor.matmul(vp[:], emb8_sb[:, 2 * k2:2 * k2 + 2, sm * 128:(sm + 1) * 128],
                                                 wv_sb[:, 2 * k2:2 * k2 + 2, nn2 * 512:(nn2 + 1) * 512],
                                                 start=k2 == 0, stop=k2 == 3, perf_mode=DR)
                            for j in range(4):
                                nc.scalar.activation(
                                    vd_sb[:, 4 * nn2 + j, sm, :],
                                    vp[:, j * 128:(j + 1) * 128],
                                    ACTF.Copy, scale=DS_QKV)

                # ---------- phase 2: attention ----------
                with tc.tile_pool(name="ph2", bufs=2) as ph2, \
                     tc.tile_pool(name="ph2c", bufs=1) as ph2c, \
                     tc.tile_pool(name="ph2p", bufs=2) as ph2p:
                    maskT_sb = ph2c.tile([128, 16, QL], BF)
                    nc.sync.dma_start(out=maskT_sb[:], in_=maskT[:])
                    dmask_sb = ph2c.tile([128, 128], F32)
                    nc.sync.dma_start(out=dmask_sb[:], in_=dmask[:])
                    wo_sb = ph2c.tile([128, 8, 1024], BF)
                    nc.sync.dma_start(out=wo_sb[:], in_=woT[:])
                    outT = ph2c.tile([128, 8, QL], BF)

                    for h in range(H):
                        khA = ph2.tile([128, 4, 512], BF, tag="khA")
                        vhA = ph2.tile([128, 16, 128], BF, tag="vhA")
                        for g in range(RPG):
                            nc.sync.dma_start(out=khA[:, g, :], in_=kvcc_out[g, :, h, :])
                            nc.sync.dma_start(out=vhA[:, 4 * g:4 * g + 4, :],
                                              in_=kvcc_out[g, :, 8 + h, :])
                        pT = ph2p.tile([128, 20, QL], BF, tag="pT")
                        for c in range(16):
                            sp = ps_b.tile([128, QL], F32, tag="sc")
                            nc.tensor.matmul(sp[:], khA[:, c // 4, (c % 4) * 128:(c % 4 + 1) * 128],
                                             qTr[:, h, :])
                            nc.vector.tensor_add(sp[:], sp[:], maskT_sb[:, c, :])
                            nc.scalar.activation(pT[:, c, :], sp[:], ACTF.Exp)
                        for qt in range(4):
                            qsl = slice(qt * 128, (qt + 1) * 128)
                            spd = ps_tr.tile([128, 128], F32, tag="sd")
                            nc.tensor.matmul(spd[:], kdT[:, h, qsl], qTr[:, h, qsl])
                            nc.vector.tensor_add(spd[:], spd[:], dmask_sb[:])
                            nc.scalar.activation(pT[:, 16 + qt, qsl], spd[:], ACTF.Exp)

                        dp = ps_row.tile([1, QL], F32, tag="row")
                        for c in range(16):
                            nc.tensor.matmul(dp[:], ones_bf[:], pT[:, c, :],
                                             start=c == 0, stop=False)
                        for qt in range(4):
                            qsl = slice(qt * 128, (qt + 1) * 128)
                            nc.tensor.matmul(dp[:, qsl], ones_bf[:], pT[:, 16 + qt, qsl],
                                             start=False, stop=qt == 3)
                        den_sb = ph2.tile([1, QL], F32, tag="den_sb")
                        nc.vector.reciprocal(den_sb[:], dp[:])
                        nc.sync.dma_start(out=den_d[h, :], in_=den_sb[0:1, :])
                        den_bc = ph2.tile([128, QL], F32, tag="den_bc")
                        row = den_d[h:h + 1, :]
                        bc_ap = bass.AP(tensor=row.tensor, offset=row.offset,
                                        ap=[[0, 128]] + list(row.ap)[1:])
                        nc.sync.dma_start(out=den_bc[:], in_=bc_ap)

                        ap_ps = ps_a.tile([128, QL], F32, tag="mm")
                        for c in range(16):
                            nc.tensor.matmul(ap_ps[:], vhA[:, c, :], pT[:, c, :],
                                             start=c == 0, stop=False)
                        for qt in range(4):
                            qsl = slice(qt * 128, (qt + 1) * 128)
                            nc.tensor.matmul(ap_ps[:, qsl], vd_sb[:, h, qt, :],
                                             pT[:, 16 + qt, qsl],
                                             start=False, stop=qt == 3)
                        nc.vector.tensor_mul(outT[:, h, :], ap_ps[:], den_bc[:])

                    # wo projection + residual + quantize + sumsq/rstd
                    h_sb = ph2c.tile([128, 8, QL], F32)
                    sqp = ps_row.tile([1, QL], F32, tag="row")
                    for mb in range(8):
                        op = ps_a.tile([128, QL], F32, tag="mm")
                        for kb in range(8):
                            nc.tensor.matmul(op[:], wo_sb[:, kb, mb * 128:(mb + 1) * 128],
                                             outT[:, kb, :], start=kb == 0, stop=kb == 7)
                        nc.vector.tensor_add(h_sb[:, mb, :], op[:], embT_sb[:, mb, :])
                        nc.scalar.activation(hq[:, mb, :], h_sb[:, mb, :], ACTF.Copy,
                                             scale=SH)
                        hsq = ph2.tile([128, QL], F32, tag="hsq")
                        nc.vector.tensor_mul(hsq[:], h_sb[:, mb, :], h_sb[:, mb, :])
                        nc.tensor.matmul(sqp[:], ones_f32[:], hsq[:],
                                         start=mb == 0, stop=mb == 7)
                    sq_sb = ph2.tile([1, QL], F32, tag="sq_sb")
                    nc.scalar.activation(sq_sb[:], sqp[:], ACTF.Sqrt,
                                         bias=eps_t[0:1, :], scale=1.0 / D)
                    rstd_sb = ph2.tile([1, QL], F32, tag="rstd_sb")
                    nc.vector.reciprocal(rstd_sb[:], sq_sb[:])
                    nc.sync.dma_start(out=rstd_d[:], in_=rstd_sb[0:1, :])
                    nc.sync.dma_start(out=stats[3:4, :], in_=rstd_sb[:])

                # ---------- phase 3: LM head (fp8 DoubleRow) ----------
                with tc.tile_pool(name="ph3", bufs=2) as ph3, \
                     tc.tile_pool(name="ph3c", bufs=1) as ph3c, \
                     tc.tile_pool(name="ph3a", bufs=1) as ph3a:
                    lbl_sb = ph3c.tile([128, 8, QL], FP8)
                    nc.sync.dma_start(out=lbl_sb[:], in_=lbl8[:])
                    idf_sb = ph3c.tile([128, 128], F32)
                    nc.sync.dma_start(out=idf_sb[:], in_=ident_f32[:])
                    m0 = [ph3a.tile([128, 1], F32, name=f"m0_{qt}") for qt in range(4)]
                    se = [ph3a.tile([128, 1], F32, name=f"se_{qt}") for qt in range(4)]
                    rstd2 = [ph3a.tile([128, 1], F32, name=f"rstd2_{qt}") for qt in range(4)]
                    for qt in range(4):
                        nc.vector.memset(m0[qt][:], 0.0)
                        nc.vector.memset(se[qt][:], 0.0)
                        rp = ph3a.tile([128, 1], F32, name=f"rp_{qt}")
                        nc.sync.dma_start(out=rp[:], in_=rstd_d[qt * 128:(qt + 1) * 128])
                        nc.vector.tensor_scalar_mul(rstd2[qt][:], rp[:], DS_LM)

                    for vg in range(NVG):
                        ncol = min(VG, V - vg * VG)
                        nch = (ncol + 511) // 512
                        lmt = ph3.tile([128, 8, VG], FP8, tag="lmt")
                        nc.sync.dma_start(out=lmt[:, :, :ncol],
                                          in_=lm8[:, :, vg * VG:vg * VG + ncol])
                        for qt in range(4):
                            qsl = slice(qt * 128, (qt + 1) * 128)
                            pss = [ps_a.tile([128, 512], F32, tag="mm",
                                             name=f"lp_{vg}_{qt}_{i}") for i in range(nch)]
                            for k2 in range(4):
                                for cc in range(nch):
                                    w = min(512, ncol - cc * 512)
                                    nc.tensor.matmul(pss[cc][:, :w],
                                                     hq[:, 2 * k2:2 * k2 + 2, qsl],
                                                     lmt[:, 2 * k2:2 * k2 + 2, cc * 512:cc * 512 + w],
                                                     start=k2 == 0, stop=k2 == 3,
                                                     perf_mode=DR)
                            for cc in range(nch):
                                w = min(512, ncol - cc * 512)
                                ej = ph3.tile([128, 512], BF, tag="ej")
                                csum = ph3.tile([128, 1], F32, tag="csum")
                                nc.scalar.activation(ej[:, :w], pss[cc][:, :w], ACTF.Exp,
                                                     scale=rstd2[qt][:], accum_out=csum[:])
                                nc.vector.tensor_add(se[qt][:], se[qt][:], csum[:])
                                cmax = ph3.tile([128, 1], F32, tag="cmax")
                                nc.vector.tensor_reduce(cmax[:], ej[:, :w], axis=AX.X, op=OP.max)
                                nc.vector.tensor_max(m0[qt][:], m0[qt][:], cmax[:])

                    for qt in range(4):
                        qsl = slice(qt * 128, (qt + 1) * 128)
                        lpp = ps_tr.tile([128, 128], F32, tag="sd")
                        for k2 in range(4):
                            nc.tensor.matmul(lpp[:], hq[:, 2 * k2:2 * k2 + 2, qsl],
                                             lbl_sb[:, 2 * k2:2 * k2 + 2, qsl],
                                             start=k2 == 0, stop=k2 == 3, perf_mode=DR)
                        junk = ph3.tile([128, 128], F32, tag="junk")
                        labd = ph3.tile([128, 1], F32, tag="labd")
                        nc.vector.tensor_mul(junk[:], lpp[:], idf_sb[:])
                        nc.vector.tensor_reduce(labd[:], junk[:], axis=AX.X, op=OP.add)
                        nc.sync.dma_start(out=stats[1:2, qsl], in_=labd[:])
                        nc.sync.dma_start(out=stats[0:1, qsl], in_=m0[qt][:])
                        nc.sync.dma_start(out=stats[2:3, qsl], in_=se[qt][:])
    nc.compile()
    _PROG = (nc, names)
    return _PROG


def kernel(**inputs):
    import os
    from concourse.bass_utils import run_bass_kernel_spmd
    nc, names = build_program()
    per_core, meta = host_prep(inputs)
    in_maps = []
    for c in range(NCORES):
        m = {}
        for k, arr in per_core[c].items():
            if arr.dtype == BF16 or arr.dtype == F8E4:
                m[names[k]] = np.ascontiguousarray(arr)
            else:
                m[names[k]] = np.ascontiguousarray(arr, dtype=np.float32)
        in_maps.append(m)
    trace = bool(os.environ.get("DFLASH_TRACE"))
    if trace:
        try:
            import ntff_shim
            ntff_shim.install()
        except Exception:
            trace = False
    res = run_bass_kernel_spmd(nc, in_maps, list(range(NCORES)), trace=trace)
    kernel.last_exec_ns = res.exec_time_ns
    stats_list = [res.results[c][names["stats"]] for c in range(NCORES)]
    return host_reduce(stats_list, meta)
